# revision 3
# baseline (speedup 1.0000x reference)
"""Trainium2 Bass kernel for nn_ConvSplitTree.

Math (see reference):
  value = sigmoid(conv2d(x, wnorm))                [N,6,H,W]
  leaf  = sum_d (value_d < 0.6) * 2^(5-d)          [N,H,W]
  pred  = conv2d(data, w_pred) + b_pred            [N,64,H,W]
  y     = sum_c pred_c / 64 + pred[leaf]           [N,H,W]

Sharding: 8 shards = 4 samples x 2 image halves (256 rows each), data
parallel, 1-row halo handled by host-side zero padding to [HC+2, 514].

Per-core kernel layout:
  - pred conv: channel-major matmuls. K=97 partitions = (3 dy x 32 cin)
    + ones row (bias), M=64 output channels, N=512 pixels. 3 dx-shifted
    fp32r matmuls per output row accumulate SAME conv in PSUM. Two rows
    stacked per [128,512] PSUM tile.
  - split conv: 16 rows block-diagonally packed: K=48=(3 dy x 16 rows),
    M=96=(16 rows x 6 ch), fp32 for exactness of the routing bits.
  - bits = (value < THR) on DVE, THR = fp32 boundary of sigmoid(v)<0.6.
  - leaf broadcast to both 64-channel halves via block-diagonal powers
    matmul (exact in fp32r: all quantities are small integers).
  - m = (leaf == chan_idx) + 1/64 (one DVE tensor_scalar op)
  - t = pred * m (DVE), y(2 rows) = ones2.T @ t (one fp32r matmul doing
    the 64->1 channel reduction for both rows: equals base + sel).
"""

import os
import sys

import numpy as np

for _p in ("/opt/trn_rl_repo", "/root/.axon_site/_ro/trn_rl_repo"):
    if os.path.isdir(_p) and _p not in sys.path:
        sys.path.insert(0, _p)

from contextlib import ExitStack

import concourse.bacc as bacc
import concourse.tile as tile
from concourse import mybir
from concourse.bass_utils import run_bass_kernel_spmd

N_CORES = 8
N, H, W = 4, 512, 512
HC = H // 2          # rows per core
WP = W + 2           # padded width
CIN, COUT, D = 32, 64, 6
RG = 16              # rows per group
NG = HC // RG        # groups per core
NB = RG // 2         # 2-row blocks per group
# fp32 boundary: (v < THR) == (float32_sigmoid(v) < 0.6) for all fp32 v
THR = float(np.float32(0.4054651))

F32 = mybir.dt.float32
F32R = mybir.dt.float32r

_PROGRAM = None
LAST_RESULT = None


def _build_program():
    nc = bacc.Bacc(
        "TRN2", target_bir_lowering=False, debug=False, enable_asserts=False
    )
    data_d = nc.dram_tensor("data_pad", [CIN, HC + 2, WP], F32, kind="ExternalInput").ap()
    x_d = nc.dram_tensor("x_pad", [HC + 2, WP], F32, kind="ExternalInput").ap()
    wsplit_d = nc.dram_tensor("wsplit_t", [48, 3, 96], F32, kind="ExternalInput").ap()
    wpred_d = nc.dram_tensor("wpred_t", [97, 3, 64], F32, kind="ExternalInput").ap()
    leafpow_d = nc.dram_tensor("leafpow", [96, NB, 128], F32, kind="ExternalInput").ap()
    ones2_d = nc.dram_tensor("ones2", [128, 2], F32, kind="ExternalInput").ap()
    cidx_d = nc.dram_tensor("cidx", [128, 1], F32, kind="ExternalInput").ap()
    onesrow_d = nc.dram_tensor("onesrow", [1, RG, WP], F32, kind="ExternalInput").ap()
    y_d = nc.dram_tensor("y", [HC, W], F32, kind="ExternalOutput").ap()

    lt = mybir.AluOpType.is_lt
    eq = mybir.AluOpType.is_equal
    add = mybir.AluOpType.add
    mult = mybir.AluOpType.mult

    with tile.TileContext(nc) as tc, ExitStack() as ctx:
        consts = ctx.enter_context(tc.tile_pool(name="consts", bufs=1))
        s_pool = ctx.enter_context(tc.tile_pool(name="s", bufs=2))
        x_pool = ctx.enter_context(tc.tile_pool(name="x", bufs=2))
        b_pool = ctx.enter_context(tc.tile_pool(name="bits", bufs=2))
        w_pool = ctx.enter_context(tc.tile_pool(name="work", bufs=3))
        o_pool = ctx.enter_context(tc.tile_pool(name="ystage", bufs=2))
        ps_val = ctx.enter_context(tc.tile_pool(name="ps_val", bufs=2, space="PSUM"))
        ps_pred = ctx.enter_context(tc.tile_pool(name="ps_pred", bufs=2, space="PSUM"))
        ps_leaf = ctx.enter_context(tc.tile_pool(name="ps_leaf", bufs=2, space="PSUM"))
        ps_y = ctx.enter_context(tc.tile_pool(name="ps_y", bufs=2, space="PSUM"))

        wsplit_t = consts.tile([48, 3, 96], F32)
        nc.sync.dma_start(out=wsplit_t, in_=wsplit_d)
        wpred_t = consts.tile([97, 3, 64], F32)
        nc.sync.dma_start(out=wpred_t, in_=wpred_d)
        leafpow_t = consts.tile([96, NB, 128], F32)
        nc.sync.dma_start(out=leafpow_t, in_=leafpow_d)
        ones2_t = consts.tile([128, 2], F32)
        nc.sync.dma_start(out=ones2_t, in_=ones2_d)
        cidx_t = consts.tile([128, 1], F32)
        nc.sync.dma_start(out=cidx_t, in_=cidx_d)

        for g in range(NG):
            r0 = g * RG
            # data, stacked by dy: partition (dy*32+ci), row l = padded row r0+dy+l
            S = s_pool.tile([97, RG, WP], F32, tag="S")
            for dy in range(3):
                nc.sync.dma_start(
                    out=S[32 * dy : 32 * dy + 32, :, :],
                    in_=data_d[:, r0 + dy : r0 + dy + RG, :],
                )
            nc.sync.dma_start(out=S[96:97, :, :], in_=onesrow_d)  # bias row

            # x rows, stacked by dy: partition (dy*16+g'), = padded row r0+dy+g'
            xT = x_pool.tile([48, WP], F32, tag="xT")
            for dy in range(3):
                nc.sync.dma_start(
                    out=xT[16 * dy : 16 * dy + 16, :],
                    in_=x_d[r0 + dy : r0 + dy + RG, :],
                )

            # split conv, 16 rows at once: out partition (g'*6+d)
            val = ps_val.tile([96, W], F32, tag="val")
            for dx in range(3):
                nc.tensor.matmul(
                    val,
                    lhsT=wsplit_t[:, dx, :],
                    rhs=xT[:, dx : dx + W],
                    start=(dx == 0),
                    stop=(dx == 2),
                )
            bits = b_pool.tile([96, W], F32, tag="bits")
            nc.vector.tensor_scalar(
                out=bits, in0=val, scalar1=THR, scalar2=None, op0=lt
            )

            # partition 0 = even rows, partition 1 = odd rows (engines can
            # only write at partition starts 0/32/64/96)
            ystage = o_pool.tile([2, NB, W], F32, tag="ystage")
            for j in range(NB):
                pred = ps_pred.tile([128, W], F32, tag="pred")
                for half in range(2):
                    l = 2 * j + half
                    for dx in range(3):
                        nc.tensor.matmul(
                            pred[64 * half : 64 * half + 64, :],
                            lhsT=wpred_t[:, dx, :].bitcast(F32R),
                            rhs=S[:, l, dx : dx + W].bitcast(F32R),
                            start=(dx == 0),
                            stop=(dx == 2),
                        )
                # leaf value broadcast to both 64-partition halves (exact ints)
                leafb = ps_leaf.tile([128, W], F32, tag="leafb")
                nc.tensor.matmul(
                    leafb,
                    lhsT=leafpow_t[:, j, :].bitcast(F32R),
                    rhs=bits[:, :].bitcast(F32R),
                    start=True,
                    stop=True,
                )
                m = w_pool.tile([128, W], F32, tag="m")
                nc.vector.tensor_scalar(
                    out=m, in0=leafb, scalar1=cidx_t, scalar2=1.0 / 64,
                    op0=eq, op1=add,
                )
                t = w_pool.tile([128, W], F32, tag="t")
                nc.vector.tensor_tensor(out=t, in0=pred, in1=m, op=mult)
                y2 = ps_y.tile([2, W], F32, tag="y2")
                nc.tensor.matmul(
                    y2,
                    lhsT=ones2_t[:, :].bitcast(F32R),
                    rhs=t[:, :].bitcast(F32R),
                    start=True,
                    stop=True,
                )
                nc.scalar.copy(ystage[:, j, :], y2)
            nc.sync.dma_start(
                out=y_d[r0 : r0 + RG, :].rearrange("(j two) w -> two j w", two=2),
                in_=ystage,
            )
    return nc


def _consts(w_split, w_pred, b_pred):
    # normalize split weights exactly like the reference (fp32 ops)
    w = np.maximum(w_split.astype(np.float32), np.float32(0.0))
    s = w.sum(axis=(1, 2, 3), keepdims=True, dtype=np.float32)
    wn = np.where(s < np.float32(0.1), w + np.float32(0.1 / 9.0), w)

    wsplit_t = np.zeros((48, 3, 96), np.float32)
    for dy in range(3):
        for gg in range(RG):
            for d in range(D):
                wsplit_t[dy * 16 + gg, :, gg * 6 + d] = wn[d, 0, dy, :]

    wpred_t = np.zeros((97, 3, 64), np.float32)
    # [co, ci, dy, dx] -> [(dy*32+ci), dx, co]
    wpred_t[:96] = np.ascontiguousarray(
        w_pred.astype(np.float32).transpose(2, 1, 3, 0)
    ).reshape(96, 3, 64)
    wpred_t[96, 0, :] = b_pred.astype(np.float32)

    pw = (2.0 ** np.arange(5, -1, -1)).astype(np.float32)
    leafpow = np.zeros((96, NB, 128), np.float32)
    for j in range(NB):
        for d in range(D):
            leafpow[(2 * j) * 6 + d, j, 0:64] = pw[d]
            leafpow[(2 * j + 1) * 6 + d, j, 64:128] = pw[d]

    ones2 = np.zeros((128, 2), np.float32)
    ones2[:64, 0] = 1.0
    ones2[64:, 1] = 1.0
    cidx = (np.arange(128) % 64).astype(np.float32)[:, None]
    onesrow = np.ones((1, RG, WP), np.float32)
    return {
        "wsplit_t": wsplit_t,
        "wpred_t": wpred_t,
        "leafpow": leafpow,
        "ones2": ones2,
        "cidx": cidx,
        "onesrow": onesrow,
    }


def make_in_maps(x, data, w_split, w_pred, b_pred):
    x = np.asarray(x, np.float32)
    data = np.asarray(data, np.float32)
    consts = _consts(np.asarray(w_split), np.asarray(w_pred), np.asarray(b_pred))

    xp = np.zeros((N, H + 2, WP), np.float32)
    xp[:, 1 : H + 1, 1 : W + 1] = x[:, 0]
    dp = np.zeros((N, CIN, H + 2, WP), np.float32)
    dp[:, :, 1 : H + 1, 1 : W + 1] = data

    in_maps = []
    for c in range(N_CORES):
        n, half = divmod(c, 2)
        r0 = half * HC
        m = dict(consts)
        m["data_pad"] = np.ascontiguousarray(dp[n, :, r0 : r0 + HC + 2, :])
        m["x_pad"] = np.ascontiguousarray(xp[n, r0 : r0 + HC + 2, :])
        in_maps.append(m)
    return in_maps


def kernel(x, data, w_split, w_pred, b_pred):
    global _PROGRAM, LAST_RESULT
    if _PROGRAM is None:
        _PROGRAM = _build_program()
    in_maps = make_in_maps(x, data, w_split, w_pred, b_pred)
    res = run_bass_kernel_spmd(_PROGRAM, in_maps, list(range(N_CORES)))
    LAST_RESULT = res
    y = np.empty((N, H, W), np.float32)
    for c in range(N_CORES):
        n, half = divmod(c, 2)
        y[n, half * HC : (half + 1) * HC, :] = res.results[c]["y"]
    return y


# revision 4
# speedup vs baseline: 3.4261x; 3.4261x over previous
"""Trainium2 Bass kernel for nn_ConvSplitTree.

Math (see reference):
  value = sigmoid(conv2d(x, wnorm))                [N,6,H,W]
  leaf  = sum_d (value_d < 0.6) * 2^(5-d)          [N,H,W]
  pred  = conv2d(data, w_pred) + b_pred            [N,64,H,W]
  y     = sum_c pred_c / 64 + pred[leaf]           [N,H,W]

Sharding: 8 shards = 4 samples x 2 image halves (256 rows each), data
parallel, 1-row halo handled by host-side zero padding to [HC+2, 514].

Per-core kernel layout:
  - pred conv: channel-major matmuls. K=96 partitions = (3 dy x 32 cin),
    M=64 output channels, N=512 pixels. 3 dx-shifted fp32r matmuls per
    output row accumulate the SAME-conv in PSUM; two rows share one
    [128,512] PSUM tile. The dy-stacked data tile is filled by a single
    96-partition DMA whose source access pattern reads the three
    overlapping row windows of each channel plane.
  - split conv: 16 rows block-diagonally packed in one matmul chain:
    K=48=(3 dy x 16 rows), M=96=(16 rows x 6 ch), fp32 for exactness of
    the routing bits.
  - bits = (value < THR) on DVE; THR = fp32 boundary of sigmoid(v)<0.6.
  - leaf broadcast to both 64-channel halves via block-diagonal powers
    matmul (exact in fp32r: all quantities are small integers).
  - m = (leaf == chan_idx) + 1/64 (one DVE tensor_scalar op)
  - t = (pred + b) * m (one DVE scalar_tensor_tensor op)
  - y(2 rows) = ones2.T @ t: one fp32r matmul does the 64->1 channel
    reduction for both rows and equals base + sel.
"""

import os
import sys

import numpy as np

for _p in ("/opt/trn_rl_repo", "/root/.axon_site/_ro/trn_rl_repo"):
    if os.path.isdir(_p) and _p not in sys.path:
        sys.path.insert(0, _p)

from contextlib import ExitStack

import concourse.bacc as bacc
import concourse.bass as bass
import concourse.tile as tile
from concourse import mybir
from concourse.bass_utils import run_bass_kernel_spmd

N_CORES = 8
N, H, W = 4, 512, 512
HC = H // 2          # rows per core
WP = W + 2           # padded width
CIN, COUT, D = 32, 64, 6
RG = 16              # rows per group
NG = HC // RG        # groups per core
NB = RG // 2         # 2-row blocks per group
# fp32 boundary: (v < THR) == (float32_sigmoid(v) < 0.6) for all fp32 v
THR = float(np.float32(0.4054651))

F32 = mybir.dt.float32
F32R = mybir.dt.float32r

_PROGRAM = None
LAST_RESULT = None


def _build_program():
    nc = bacc.Bacc(
        "TRN2", target_bir_lowering=False, debug=False, enable_asserts=False
    )
    data_d = nc.dram_tensor("data_pad", [CIN, HC + 2, WP], F32, kind="ExternalInput").ap()
    x_d = nc.dram_tensor("x_pad", [HC + 2, WP], F32, kind="ExternalInput").ap()
    wsplit_d = nc.dram_tensor("wsplit_t", [48, 3, 96], F32, kind="ExternalInput").ap()
    wpred_d = nc.dram_tensor("wpred_t", [96, 3, 64], F32, kind="ExternalInput").ap()
    leafpow_d = nc.dram_tensor("leafpow", [96, NB, 128], F32, kind="ExternalInput").ap()
    ones2_d = nc.dram_tensor("ones2", [128, 2], F32, kind="ExternalInput").ap()
    cidx_d = nc.dram_tensor("cidx", [128, 1], F32, kind="ExternalInput").ap()
    bvec_d = nc.dram_tensor("bvec", [128, 1], F32, kind="ExternalInput").ap()
    y_d = nc.dram_tensor("y", [HC, W], F32, kind="ExternalOutput").ap()

    lt = mybir.AluOpType.is_lt
    eq = mybir.AluOpType.is_equal
    add = mybir.AluOpType.add
    mult = mybir.AluOpType.mult

    with tile.TileContext(nc) as tc, ExitStack() as ctx:
        consts = ctx.enter_context(tc.tile_pool(name="consts", bufs=1))
        s_pool = ctx.enter_context(tc.tile_pool(name="s", bufs=2))
        x_pool = ctx.enter_context(tc.tile_pool(name="x", bufs=2))
        b_pool = ctx.enter_context(tc.tile_pool(name="bits", bufs=2))
        w_pool = ctx.enter_context(tc.tile_pool(name="work", bufs=3))
        o_pool = ctx.enter_context(tc.tile_pool(name="ystage", bufs=2))
        ps_val = ctx.enter_context(tc.tile_pool(name="ps_val", bufs=2, space="PSUM"))
        ps_pred = ctx.enter_context(tc.tile_pool(name="ps_pred", bufs=2, space="PSUM"))
        ps_leaf = ctx.enter_context(tc.tile_pool(name="ps_leaf", bufs=2, space="PSUM"))
        ps_y = ctx.enter_context(tc.tile_pool(name="ps_y", bufs=2, space="PSUM"))

        wsplit_t = consts.tile([48, 3, 96], F32)
        nc.scalar.dma_start(out=wsplit_t, in_=wsplit_d)
        wpred_t = consts.tile([96, 3, 64], F32)
        nc.scalar.dma_start(out=wpred_t, in_=wpred_d)
        leafpow_t = consts.tile([96, NB, 128], F32)
        nc.scalar.dma_start(out=leafpow_t, in_=leafpow_d)
        ones2_t = consts.tile([128, 2], F32)
        nc.scalar.dma_start(out=ones2_t, in_=ones2_d)
        cidx_t = consts.tile([128, 1], F32)
        nc.scalar.dma_start(out=cidx_t, in_=cidx_d)
        bvec_t = consts.tile([128, 1], F32)
        nc.scalar.dma_start(out=bvec_t, in_=bvec_d)

        for g in range(NG):
            r0 = g * RG
            # data, stacked by dy: partition (dy*32+ci), row l = padded row
            # r0+dy+l. One 96-partition DMA; the source AP walks the three
            # overlapping row windows of every channel plane.
            S = s_pool.tile([96, RG, WP], F32, tag="S")
            src = bass.AP(
                tensor=data_d.tensor,
                offset=r0 * WP,
                ap=[
                    [WP, 3],              # dy: row shift
                    [(HC + 2) * WP, CIN], # ci: channel plane
                    [WP, RG],             # l: row within group
                    [1, WP],              # col
                ],
            )
            nc.sync.dma_start(out=S, in_=src)

            # x rows, stacked by dy: partition (dy*16+g') = padded row r0+dy+g'
            xT = x_pool.tile([48, WP], F32, tag="xT")
            for dy in range(3):
                nc.scalar.dma_start(
                    out=xT[16 * dy : 16 * dy + 16, :],
                    in_=x_d[r0 + dy : r0 + dy + RG, :],
                )

            # split conv, 16 rows at once: out partition (g'*6+d)
            val = ps_val.tile([96, W], F32, tag="val")
            for dx in range(3):
                nc.tensor.matmul(
                    val,
                    lhsT=wsplit_t[:, dx, :],
                    rhs=xT[:, dx : dx + W],
                    start=(dx == 0),
                    stop=(dx == 2),
                )
            bits = b_pool.tile([96, W], F32, tag="bits")
            nc.vector.tensor_scalar(
                out=bits, in0=val, scalar1=THR, scalar2=None, op0=lt
            )

            # output staging: block j -> partitions (j%4)*32+{0,1}, slot j//4
            ystage = o_pool.tile([128, NB // 4, W], F32, tag="ystage")
            for j in range(NB):
                pred = ps_pred.tile([128, W], F32, tag="pred")
                for half in range(2):
                    l = 2 * j + half
                    for dx in range(3):
                        nc.tensor.matmul(
                            pred[64 * half : 64 * half + 64, :],
                            lhsT=wpred_t[:, dx, :].bitcast(F32R),
                            rhs=S[:, l, dx : dx + W].bitcast(F32R),
                            start=(dx == 0),
                            stop=(dx == 2),
                        )
                # leaf value broadcast to both 64-partition halves (exact ints)
                leafb = ps_leaf.tile([128, W], F32, tag="leafb")
                nc.tensor.matmul(
                    leafb,
                    lhsT=leafpow_t[:, j, :].bitcast(F32R),
                    rhs=bits[:, :].bitcast(F32R),
                    start=True,
                    stop=True,
                )
                m = w_pool.tile([128, W], F32, tag="m")
                nc.vector.tensor_scalar(
                    out=m, in0=leafb, scalar1=cidx_t, scalar2=1.0 / 64,
                    op0=eq, op1=add,
                )
                # t = (pred + b) * m
                t = w_pool.tile([128, W], F32, tag="t")
                nc.vector.scalar_tensor_tensor(
                    out=t, in0=pred, scalar=bvec_t, in1=m, op0=add, op1=mult
                )
                y2 = ps_y.tile([2, W], F32, tag="y2")
                nc.tensor.matmul(
                    y2,
                    lhsT=ones2_t[:, :].bitcast(F32R),
                    rhs=t[:, :].bitcast(F32R),
                    start=True,
                    stop=True,
                )
                q, f = j % 4, j // 4
                nc.scalar.copy(ystage[q * 32 : q * 32 + 2, f, :], y2)
            # rows r0 + 8f + 2q + half <- ystage[q*32+half, f, :]
            y_slice = y_d[r0 : r0 + RG, :].rearrange(
                "(f q h) w -> q h f w", f=2, q=4, h=2
            )
            for q in range(4):
                nc.gpsimd.dma_start(
                    out=y_slice[q], in_=ystage[q * 32 : q * 32 + 2, :, :]
                )
    return nc


def _consts(w_split, w_pred, b_pred):
    # normalize split weights exactly like the reference (fp32 ops)
    w = np.maximum(w_split.astype(np.float32), np.float32(0.0))
    s = w.sum(axis=(1, 2, 3), keepdims=True, dtype=np.float32)
    wn = np.where(s < np.float32(0.1), w + np.float32(0.1 / 9.0), w)

    wsplit_t = np.zeros((48, 3, 96), np.float32)
    for dy in range(3):
        for gg in range(RG):
            for d in range(D):
                wsplit_t[dy * 16 + gg, :, gg * 6 + d] = wn[d, 0, dy, :]

    # [co, ci, dy, dx] -> [(dy*32+ci), dx, co]
    wpred_t = np.ascontiguousarray(
        w_pred.astype(np.float32).transpose(2, 1, 3, 0)
    ).reshape(96, 3, 64)

    pw = (2.0 ** np.arange(5, -1, -1)).astype(np.float32)
    leafpow = np.zeros((96, NB, 128), np.float32)
    for j in range(NB):
        for d in range(D):
            leafpow[(2 * j) * 6 + d, j, 0:64] = pw[d]
            leafpow[(2 * j + 1) * 6 + d, j, 64:128] = pw[d]

    ones2 = np.zeros((128, 2), np.float32)
    ones2[:64, 0] = 1.0
    ones2[64:, 1] = 1.0
    cidx = (np.arange(128) % 64).astype(np.float32)[:, None]
    bvec = np.concatenate([b_pred, b_pred]).astype(np.float32)[:, None]
    return {
        "wsplit_t": wsplit_t,
        "wpred_t": np.ascontiguousarray(wpred_t),
        "leafpow": leafpow,
        "ones2": ones2,
        "cidx": cidx,
        "bvec": bvec,
    }


def make_in_maps(x, data, w_split, w_pred, b_pred):
    x = np.asarray(x, np.float32)
    data = np.asarray(data, np.float32)
    consts = _consts(np.asarray(w_split), np.asarray(w_pred), np.asarray(b_pred))

    xp = np.zeros((N, H + 2, WP), np.float32)
    xp[:, 1 : H + 1, 1 : W + 1] = x[:, 0]
    dp = np.zeros((N, CIN, H + 2, WP), np.float32)
    dp[:, :, 1 : H + 1, 1 : W + 1] = data

    in_maps = []
    for c in range(N_CORES):
        n, half = divmod(c, 2)
        r0 = half * HC
        m = dict(consts)
        m["data_pad"] = np.ascontiguousarray(dp[n, :, r0 : r0 + HC + 2, :])
        m["x_pad"] = np.ascontiguousarray(xp[n, r0 : r0 + HC + 2, :])
        in_maps.append(m)
    return in_maps


def kernel(x, data, w_split, w_pred, b_pred):
    global _PROGRAM, LAST_RESULT
    if _PROGRAM is None:
        _PROGRAM = _build_program()
    in_maps = make_in_maps(x, data, w_split, w_pred, b_pred)
    res = run_bass_kernel_spmd(_PROGRAM, in_maps, list(range(N_CORES)))
    LAST_RESULT = res
    y = np.empty((N, H, W), np.float32)
    for c in range(N_CORES):
        n, half = divmod(c, 2)
        y[n, half * HC : (half + 1) * HC, :] = res.results[c]["y"]
    return y
